# revision 9
# baseline (speedup 1.0000x reference)
"""Single-head attention (B=4, N=2048, D=1024), scores scaled by 10.

Sharding: 8 cores = (batch, query-half). Each core computes K,V for the
full 2048-key sequence of its batch plus attention output for its 1024
queries. Key columns are permuted query-half-first per core so the SPMD
program is identical across cores (softmax/AV are permutation-invariant
over keys).

Numerics: Q/K projections and Q@K^T run as bf16 hi/lo 3-pass matmuls
(error ~2^-17, needed because the x10 score scale amplifies rounding into
the softmax exponent); V projection and P@V run single-pass fp16.
Scores layout is k-partitioned (St = K Q^T per tile) so the attention@V
matmul consumes P directly with no transposes; the per-query max is
computed via a fold + DVE 32x32 block transpose + free-dim reduce, and
max/1-over-sum rows are broadcast across partitions with rank-1 matmuls.
"""

import numpy as np
import ml_dtypes

B, SEQ, D = 4, 2048, 1024
NQ = 1024          # queries per core
QCH = 256          # attention q-chunk
NCH = NQ // QCH
NCORES = 8
DT = D // 128      # 8 d-tiles
ET = D // 128      # 8 e-tiles
KT = SEQ // 128    # 16 k-tiles

_BUILT = {}


def _build():
    if "nc" in _BUILT:
        return _BUILT["nc"]
    from contextlib import ExitStack

    import concourse.bass as bass  # noqa: F401
    import concourse.mybir as mybir
    import concourse.tile as tile
    from concourse import bacc

    dt = mybir.dt
    F32, BF, F16 = dt.float32, dt.bfloat16, dt.float16
    AL = mybir.AluOpType
    EXP = mybir.ActivationFunctionType.Exp

    nc = bacc.Bacc("TRN2", target_bir_lowering=False, debug=False)

    xh_d = nc.dram_tensor("xh", [D, SEQ], BF, kind="ExternalInput")
    xl_d = nc.dram_tensor("xl", [D, SEQ], BF, kind="ExternalInput")
    xf_d = nc.dram_tensor("xf", [D, SEQ], F16, kind="ExternalInput")
    wqh_d = nc.dram_tensor("wqh", [D, D], BF, kind="ExternalInput")
    wql_d = nc.dram_tensor("wql", [D, D], BF, kind="ExternalInput")
    wkh_d = nc.dram_tensor("wkh", [D, D], BF, kind="ExternalInput")
    wkl_d = nc.dram_tensor("wkl", [D, D], BF, kind="ExternalInput")
    wvf_d = nc.dram_tensor("wvf", [D, D], F16, kind="ExternalInput")
    ot_d = nc.dram_tensor("ot", [D, NQ], F32, kind="ExternalOutput")

    xh_r = xh_d.ap().rearrange("(t p) n -> p t n", p=128)
    xl_r = xl_d.ap().rearrange("(t p) n -> p t n", p=128)
    xf_r = xf_d.ap().rearrange("(t p) n -> p t n", p=128)
    wqh_r = wqh_d.ap().rearrange("(t p) e -> p t e", p=128)
    wql_r = wql_d.ap().rearrange("(t p) e -> p t e", p=128)
    wkh_r = wkh_d.ap().rearrange("(t p) e -> p t e", p=128)
    wkl_r = wkl_d.ap().rearrange("(t p) e -> p t e", p=128)
    wvf_r = wvf_d.ap().rearrange("(t p) e -> p t e", p=128)
    ot_r = ot_d.ap().rearrange("(t p) q -> p t q", p=128)

    with tile.TileContext(nc) as tc, ExitStack() as ctx:
        qk_pool = ctx.enter_context(tc.tile_pool(name="qk", bufs=1))
        qth = qk_pool.tile([128, ET, NQ], BF, tag="qth")
        qtl = qk_pool.tile([128, ET, NQ], BF, tag="qtl")
        kth = qk_pool.tile([128, ET, SEQ], BF, tag="kth")
        ktl = qk_pool.tile([128, ET, SEQ], BF, tag="ktl")

        const_pool = ctx.enter_context(tc.tile_pool(name="const", bufs=1))
        ones16 = const_pool.tile([128, 1], F16, tag="ones16")
        ten32 = const_pool.tile([1, 128], F32, tag="ten32")
        one32 = const_pool.tile([1, 128], F32, tag="one32")
        nc.vector.memset(ones16[:], 1.0)
        nc.vector.memset(ten32[:], 10.0)
        nc.vector.memset(one32[:], 1.0)

        # ---------------- Phase A1: Q^T, K^T projections (bf16 3-pass) ----
        with (
            tc.tile_pool(name="xhl", bufs=1) as xpool,
            tc.tile_pool(name="wstr", bufs=3) as wpool,
            tc.tile_pool(name="psA", bufs=4, space="PSUM") as psA,
        ):
            xh_t = xpool.tile([128, DT, SEQ], BF, tag="xh")
            xl_t = xpool.tile([128, DT, SEQ], BF, tag="xl")
            nc.sync.dma_start(xh_t[:], xh_r[:])
            nc.sync.dma_start(xl_t[:], xl_r[:])

            for wh_r, wl_r, outh, outl, n_all in (
                (wqh_r, wql_r, qth, qtl, NQ),
                (wkh_r, wkl_r, kth, ktl, SEQ),
            ):
                for et in range(ET):
                    e0 = 128 * et
                    wh_t = wpool.tile([128, DT, 128], BF, tag="wh")
                    wl_t = wpool.tile([128, DT, 128], BF, tag="wl")
                    for dti in range(DT):
                        nc.sync.dma_start(wh_t[:, dti, :], wh_r[:, dti, e0 : e0 + 128])
                        nc.sync.dma_start(wl_t[:, dti, :], wl_r[:, dti, e0 : e0 + 128])
                    for chn in range(n_all // 512):
                        n0 = 512 * chn
                        ps = psA.tile([128, 512], F32, tag="psA")
                        i = 0
                        for dti in range(DT):
                            for lw, rx in (
                                (wh_t, xh_t),
                                (wh_t, xl_t),
                                (wl_t, xh_t),
                            ):
                                nc.tensor.matmul(
                                    ps[:],
                                    lw[:, dti, :],
                                    rx[:, dti, n0 : n0 + 512],
                                    start=(i == 0),
                                    stop=(i == 3 * DT - 1),
                                )
                                i += 1
                        hi = outh[:, et, n0 : n0 + 512]
                        nc.vector.tensor_copy(hi, ps[:])
                        nc.vector.scalar_tensor_tensor(
                            outl[:, et, n0 : n0 + 512],
                            ps[:],
                            1.0,
                            hi,
                            op0=AL.mult,
                            op1=AL.subtract,
                        )

        # ---------------- Phase A2: V projection (fp16 1-pass) ------------
        v_pool = ctx.enter_context(tc.tile_pool(name="vp", bufs=1))
        vf = v_pool.tile([128, KT, D], F16, tag="vf")
        with (
            tc.tile_pool(name="xfp", bufs=1) as xfpool,
            tc.tile_pool(name="wvp", bufs=2) as wvpool,
            tc.tile_pool(name="psV", bufs=4, space="PSUM") as psV,
        ):
            xf_t = xfpool.tile([128, DT, SEQ], F16, tag="xf")
            nc.sync.dma_start(xf_t[:], xf_r[:])
            for ec in range(2):
                e0 = 512 * ec
                wv_t = wvpool.tile([128, DT, 512], F16, tag="wv")
                nc.sync.dma_start(wv_t[:], wvf_r[:, :, e0 : e0 + 512])
                for kt in range(KT):
                    k0 = 128 * kt
                    ps = psV.tile([128, 512], F32, tag="psV")
                    for dti in range(DT):
                        nc.tensor.matmul(
                            ps[:],
                            xf_t[:, dti, k0 : k0 + 128],
                            wv_t[:, dti, :],
                            start=(dti == 0),
                            stop=(dti == DT - 1),
                        )
                    nc.vector.tensor_copy(vf[:, kt, e0 : e0 + 512], ps[:])

        # ---------------- Phase B: attention, q-chunked -------------------
        with (
            tc.tile_pool(name="stp", bufs=2) as stpool,
            tc.tile_pool(name="pp", bufs=2) as ppool,
            tc.tile_pool(name="tree", bufs=1) as treepool,
            tc.tile_pool(name="aux", bufs=2) as auxpool,
            tc.tile_pool(name="osb", bufs=3) as outpool,
            tc.tile_pool(name="psS", bufs=3, space="PSUM") as psS,
            tc.tile_pool(name="psO", bufs=2, space="PSUM") as psO,
            tc.tile_pool(name="psX", bufs=2, space="PSUM") as psX,
            tc.tile_pool(name="psR", bufs=1, space="PSUM") as psR,
        ):
            for c in range(NCH):
                q0 = QCH * c
                st = stpool.tile([128, KT, QCH], F32, tag="st")
                for kt in range(KT):
                    k0 = 128 * kt
                    ps = psS.tile([128, QCH], F32, tag="psS")
                    i = 0
                    for et in range(ET):
                        for lK, rQ in ((kth, qth), (kth, qtl), (ktl, qth)):
                            nc.tensor.matmul(
                                ps[:],
                                lK[:, et, k0 : k0 + 128],
                                rQ[:, et, q0 : q0 + QCH],
                                start=(i == 0),
                                stop=(i == 3 * ET - 1),
                            )
                            i += 1
                    nc.vector.tensor_copy(st[:, kt, :], ps[:])

                # ---- per-query max over all keys (k lives on partitions) --
                t8 = treepool.tile([128, 8, QCH], F32, tag="t8")
                for j in range(8):
                    nc.vector.tensor_max(t8[:, j, :], st[:, 2 * j, :], st[:, 2 * j + 1, :])
                for j in range(4):
                    nc.vector.tensor_max(t8[:, j, :], t8[:, 2 * j, :], t8[:, 2 * j + 1, :])
                nc.vector.tensor_max(t8[:, 0, :], t8[:, 0, :], t8[:, 1, :])
                nc.vector.tensor_max(t8[:, 2, :], t8[:, 2, :], t8[:, 3, :])
                nc.vector.tensor_max(t8[:, 0, :], t8[:, 0, :], t8[:, 2, :])
                # fold 128 partitions -> 32: DVE ops need equal start
                # partitions, so move the four 32-partition groups with DMAs
                fold4 = treepool.tile([32, 4, QCH], F32, tag="fold4")
                for a in range(4):
                    nc.sync.dma_start(fold4[:, a, :], t8[32 * a : 32 * (a + 1), 0, :])
                nc.vector.tensor_max(fold4[:, 0, :], fold4[:, 0, :], fold4[:, 1, :])
                nc.vector.tensor_max(fold4[:, 2, :], fold4[:, 2, :], fold4[:, 3, :])
                nc.vector.tensor_max(fold4[:, 0, :], fold4[:, 0, :], fold4[:, 2, :])
                t32t = treepool.tile([32, QCH], F32, tag="t32t")
                nc.vector.transpose(t32t[:], fold4[:, 0, :])
                # mx32[r, j] = max over partitions for query q0 + 32j + r
                mx32 = treepool.tile([32, 32], F32, tag="mx32")
                nc.vector.memset(mx32[:], 0.0)
                nc.vector.reduce_max(
                    mx32[:, 0 : QCH // 32],
                    t32t[:].rearrange("p (j c) -> p j c", c=32),
                    axis=mybir.AxisListType.X,
                )
                # transpose once more so q becomes (j-part, r-free) = contiguous
                mx32t = treepool.tile([32, 32], F32, tag="mx32t")
                nc.vector.transpose(mx32t[:], mx32[:])
                m1row = treepool.tile([1, QCH], F32, tag="m1row")
                nc.sync.dma_start(m1row[:], mx32t[0 : QCH // 32, :])
                maxb_ps = psX.tile([128, QCH], F32, tag="bcast")
                nc.tensor.matmul(maxb_ps[:], ten32[:], m1row[:], start=True, stop=True)
                maxb = auxpool.tile([128, QCH], F32, tag="maxb")
                nc.vector.tensor_copy(maxb[:], maxb_ps[:])

                # ---- exp(10*s - 10*max) -> fp16 P ------------------------
                p_t = ppool.tile([128, KT, QCH], F16, tag="p")
                for kt in range(KT):
                    nc.vector.scalar_tensor_tensor(
                        st[:, kt, :],
                        st[:, kt, :],
                        10.0,
                        maxb[:],
                        op0=AL.mult,
                        op1=AL.subtract,
                    )
                    nc.scalar.activation(p_t[:, kt, :], st[:, kt, :], EXP)

                # ---- sums over keys via ones-matmul, then 1/sum ----------
                sum_ps = psR.tile([1, QCH], F32, tag="sum")
                for kt in range(KT):
                    nc.tensor.matmul(
                        sum_ps[:],
                        ones16[:],
                        p_t[:, kt, :],
                        start=(kt == 0),
                        stop=(kt == KT - 1),
                    )
                recrow = treepool.tile([1, QCH], F32, tag="recrow")
                nc.vector.reciprocal(recrow[:], sum_ps[:])
                recb_ps = psX.tile([128, QCH], F32, tag="bcast")
                nc.tensor.matmul(recb_ps[:], one32[:], recrow[:], start=True, stop=True)
                recb = auxpool.tile([128, QCH], F32, tag="recb")
                nc.vector.tensor_copy(recb[:], recb_ps[:])

                # ---- O^T[d, q] = V^T P, scaled by 1/sum ------------------
                for dti in range(DT):
                    d0 = 128 * dti
                    ops = psO.tile([128, QCH], F32, tag="psO")
                    for kt in range(KT):
                        nc.tensor.matmul(
                            ops[:],
                            vf[:, kt, d0 : d0 + 128],
                            p_t[:, kt, :],
                            start=(kt == 0),
                            stop=(kt == KT - 1),
                        )
                    osb = outpool.tile([128, QCH], F32, tag="osb")
                    nc.vector.scalar_tensor_tensor(
                        osb[:], ops[:], 1.0, recb[:], op0=AL.mult, op1=AL.mult
                    )
                    nc.sync.dma_start(ot_r[:, dti, q0 : q0 + QCH], osb[:])

    nc.compile()
    _BUILT["nc"] = nc
    return nc


def _prep_inputs(x, q_w, k_w, v_w):
    bf = ml_dtypes.bfloat16
    f16 = np.float16

    def hl(a):
        h = a.astype(bf)
        l_ = (a - h.astype(np.float32)).astype(bf)
        return h, l_

    wqh, wql = hl(np.ascontiguousarray(q_w.T))
    wkh, wkl = hl(np.ascontiguousarray(k_w.T))
    wvf = np.ascontiguousarray(v_w.T).astype(f16)

    in_maps = []
    for core in range(NCORES):
        b, h = divmod(core, 2)
        xb = np.asarray(x[b])  # [SEQ, D]
        xq = xb[NQ * h : NQ * (h + 1)]
        xo = xb[NQ * (1 - h) : NQ * (2 - h)]
        xt = np.concatenate([xq.T, xo.T], axis=1)  # [D, SEQ] queries first
        xt = np.ascontiguousarray(xt)
        xh, xl = hl(xt)
        in_maps.append(
            {
                "xh": xh,
                "xl": xl,
                "xf": xt.astype(f16),
                "wqh": wqh,
                "wql": wql,
                "wkh": wkh,
                "wkl": wkl,
                "wvf": wvf,
            }
        )
    return in_maps


def run(x, q_w, k_w, v_w, trace=False):
    from concourse.bass_utils import run_bass_kernel_spmd

    nc = _build()
    in_maps = _prep_inputs(x, q_w, k_w, v_w)
    res = run_bass_kernel_spmd(nc, in_maps, list(range(NCORES)), trace=trace)
    out = np.empty((B, SEQ, D), np.float32)
    for core in range(NCORES):
        b, h = divmod(core, 2)
        out[b, NQ * h : NQ * (h + 1)] = res.results[core]["ot"].T
    return out, res


def kernel(x, q_w, k_w, v_w):
    x = np.asarray(x, np.float32)
    q_w = np.asarray(q_w, np.float32)
    k_w = np.asarray(k_w, np.float32)
    v_w = np.asarray(v_w, np.float32)
    out, _ = run(x, q_w, k_w, v_w, trace=False)
    return out
